# revision 4
# baseline (speedup 1.0000x reference)
"""Trainium2 Bass kernel for nn_EncodingLayer (LIF encoding layer).

reference:
    drive = einsum('bsih,ih->bsh', x, encoding)        # x: [32,128,64,1024]
    repeat each drive step T=4 times, scan LIF:
        v' = 0.9*v + d;  spike = sigmoid(5*(v'-1));  v = v' - spike
    returns (spikes [32,512,1024], v_final [32,1024])

Distribution: data-parallel over batch B=32 across 8 cores (4 b each);
encoding replicated.

Per-core program (all fp32):
  Phase A (per s-chunk of 32, per b, per i-chunk of 4):
    DMA x tile [p=(s32,i4), f=h1024]; DVE multiply by enc replica;
    PE ones-matmul reduces i into psum drive [p=(b4,s32), f=h1024]
    (16 accumulating matmuls per (b,chunk)).
  ACT copies psum -> sbuf, PE transposes 32x128 blocks -> psum [h',s],
  ACT copies into scan-layout drive buffer Dscan [p=h', f=(s, b, k)].
  Scan (128 steps per chunk, interleaved with next chunk's phase A):
    vn = (v*0.9)+d (DVE stt); spike = sigmoid(5*vn-5) (ACT, writes
    output buffer); v = vn - spike (DVE).
  Output buffer [p=h', f=(st_loc, b, k)] DMAd per chunk; host reassembles.
"""

import numpy as np

B, S, I, H = 32, 128, 64, 1024
T = 4
DECAY, THRESH, BETA = 0.9, 1.0, 5.0
NCORES = 8
BL = B // NCORES          # 4 batches per core
NSC = 4                   # s-chunks
SC = S // NSC             # 32 s per chunk
NIC = 16                  # i-chunks
IC = I // NIC             # 4 i per chunk
KB = H // 128             # 8 h-blocks
ST_PER_CHUNK = SC * T     # 128 scan steps per chunk
FW = BL * KB              # 32 = free width of scan state

_PROG = None              # (nc, input names, output names)


def _build_program():
    import concourse.bass as bass
    import concourse.tile as tile
    from concourse import bacc, mybir

    f32 = mybir.dt.float32
    Alu = mybir.AluOpType

    nc = bacc.Bacc("TRN2", target_bir_lowering=False, debug=False,
                   num_devices=NCORES)

    x_d = nc.dram_tensor("x", [BL, S, I, H], f32, kind="ExternalInput")
    encreps_d = nc.dram_tensor("encreps", [NIC, 128, H], f32,
                               kind="ExternalInput")
    ones_d = nc.dram_tensor("ones32", [128, SC], f32, kind="ExternalInput")
    ident_d = nc.dram_tensor("ident32", [128, 32], f32, kind="ExternalInput")
    bias_d = nc.dram_tensor("biasm5", [128, 1], f32, kind="ExternalInput")

    out_d = nc.dram_tensor("spk", [128, S * T * FW], f32,
                           kind="ExternalOutput")  # [128, 16384]
    vf_d = nc.dram_tensor("vf", [128, FW], f32, kind="ExternalOutput")

    with tile.TileContext(nc) as tc:
        with (
            tc.tile_pool(name="consts", bufs=1) as cpool,
            tc.tile_pool(name="xin", bufs=3) as xpool,
            tc.tile_pool(name="xe", bufs=3) as xepool,
            tc.tile_pool(name="drv", bufs=2) as dpool,
            tc.tile_pool(name="dscan", bufs=3) as dscpool,
            tc.tile_pool(name="outc", bufs=2) as opool,
            tc.tile_pool(name="state", bufs=1) as vpool,
            tc.tile_pool(name="vn", bufs=2) as vnpool,
            tc.tile_pool(name="psd", bufs=2, space="PSUM") as psdpool,
            tc.tile_pool(name="pst", bufs=2, space="PSUM") as pstpool,
        ):
            # ---- constants ----
            encrep_t = []
            for ic in range(NIC):
                t_ = cpool.tile([128, H], f32, tag=f"encrep{ic}")
                nc.sync.dma_start(t_[:], encreps_d.ap()[ic])
                encrep_t.append(t_)
            ones_t = cpool.tile([128, SC], f32, tag="ones32")
            nc.sync.dma_start(ones_t[:], ones_d.ap())
            ident_t = cpool.tile([128, 32], f32, tag="ident32")
            nc.sync.dma_start(ident_t[:], ident_d.ap())
            bias_t = cpool.tile([128, 1], f32, tag="biasm5")
            nc.sync.dma_start(bias_t[:], bias_d.ap())

            v_t = vpool.tile([128, FW], f32, tag="v")
            nc.vector.memset(v_t[:], 0.0)

            # ---- per-chunk emission, scan trails phase A by one chunk ----
            dscan_tiles = [None] * NSC

            def phase_a_quads(c):
                """Yield (dma+mul+2 matmul) closures for chunk c."""
                psd = psdpool.tile([128, H], f32, tag="psd")
                for b in range(BL):
                    for ic in range(NIC):
                        def quad(b=b, ic=ic, psd=psd):
                            xt = xpool.tile([128, H], f32, tag="xin")
                            nc.sync.dma_start(
                                xt[:],
                                x_d.ap()[b, c * SC:(c + 1) * SC,
                                         ic * IC:(ic + 1) * IC, :],
                            )
                            xe = xepool.tile([128, H], f32, tag="xe")
                            nc.vector.tensor_mul(xe[:], xt[:], encrep_t[ic][:])
                            for hh in range(2):
                                nc.tensor.matmul(
                                    psd[32 * b:32 * b + 32,
                                        512 * hh:512 * hh + 512],
                                    ones_t[:],
                                    xe[:, 512 * hh:512 * hh + 512],
                                    start=(ic == 0), stop=(ic == NIC - 1),
                                    skip_group_check=True,
                                    tile_position=(0, 32 * b),
                                )
                        yield quad
                # tail: psum -> sbuf, transposes, scan-layout copies
                def tail(psd=psd, c=c):
                    drv = dpool.tile([128, H], f32, tag="drv")
                    nc.scalar.copy(drv[:], psd[:])
                    dsc = dscpool.tile([128, SC * FW], f32, tag="dscan")
                    dsc3 = dsc[:].rearrange("p (s g) -> p s g", g=FW)
                    for b in range(BL):
                        for k in range(KB):
                            pst = pstpool.tile([128, 32], f32, tag="pst")
                            nc.tensor.transpose(
                                pst[:],
                                drv[32 * b:32 * b + 32, 128 * k:128 * k + 128],
                                ident_t[32 * b:32 * b + 32, :],
                                tile_position=(32 * b, 0),
                            )
                            nc.scalar.copy(dsc3[:, :, b * KB + k], pst[:])
                    dscan_tiles[c] = dsc
                yield tail

            def scan_steps(c):
                """Yield one closure per scan step for chunk c."""
                outc = opool.tile([128, ST_PER_CHUNK * FW], f32, tag="outc")
                for sl in range(SC):
                    for t in range(T):
                        def step(sl=sl, t=t, outc=outc, c=c):
                            dsc = dscan_tiles[c]
                            d = dsc[:, sl * FW:(sl + 1) * FW]
                            vn = vnpool.tile([128, FW], f32, tag="vn")
                            nc.vector.scalar_tensor_tensor(
                                vn[:], v_t[:], DECAY, d, Alu.mult, Alu.add)
                            stl = sl * T + t
                            spk = outc[:, stl * FW:(stl + 1) * FW]
                            nc.scalar.activation(
                                spk, vn[:],
                                mybir.ActivationFunctionType.Sigmoid,
                                bias=bias_t[:], scale=BETA)
                            nc.vector.tensor_sub(v_t[:], vn[:], spk)
                        yield step
                def flush(outc=outc, c=c):
                    nc.sync.dma_start(
                        out_d.ap()[:, c * ST_PER_CHUNK * FW:
                                   (c + 1) * ST_PER_CHUNK * FW],
                        outc[:])
                yield flush

            for c in range(NSC):
                quads = list(phase_a_quads(c))
                steps = list(scan_steps(c - 1)) if c > 0 else []
                # interleave: ~1 phase-A task per 2 scan steps
                qi = si = 0
                while qi < len(quads) or si < len(steps):
                    if qi < len(quads):
                        quads[qi](); qi += 1
                    for _ in range(2):
                        if si < len(steps):
                            steps[si](); si += 1
            for f in scan_steps(NSC - 1):
                f()
            nc.sync.dma_start(vf_d.ap(), v_t[:])

    nc.compile()
    return nc


def _get_prog():
    global _PROG
    if _PROG is None:
        _PROG = _build_program()
    return _PROG


def _host_inputs(x, encoding):
    """Build the per-core in_maps."""
    enc = np.ascontiguousarray(encoding, dtype=np.float32)
    encreps = np.stack([np.tile(enc[ic * IC:(ic + 1) * IC, :], (SC, 1))
                        for ic in range(NIC)])            # [16,128,H]
    ones32 = np.zeros((128, SC), dtype=np.float32)
    ones32[np.arange(128), np.arange(128) // IC] = 1.0     # p=(s,i): p//4==s
    ident32 = np.zeros((128, 32), dtype=np.float32)
    ident32[np.arange(128), np.arange(128) % 32] = 1.0
    biasm5 = np.full((128, 1), -BETA * THRESH, dtype=np.float32)

    in_maps = []
    for core in range(NCORES):
        xs = np.ascontiguousarray(x[core * BL:(core + 1) * BL],
                                  dtype=np.float32)
        in_maps.append({
            "x": xs, "encreps": encreps, "ones32": ones32,
            "ident32": ident32, "biasm5": biasm5,
        })
    return in_maps


def kernel(x, encoding):
    from concourse import bass_utils

    nc = _get_prog()
    in_maps = _host_inputs(np.asarray(x), np.asarray(encoding))
    res = bass_utils.run_bass_kernel_spmd(nc, in_maps,
                                          core_ids=list(range(NCORES)))

    out = np.empty((B, S * T, H), dtype=np.float32)
    vf = np.empty((B, H), dtype=np.float32)
    for core in range(NCORES):
        r = res.results[core]
        spk = r["spk"].reshape(128, S * T, BL, KB)      # [h', st, b, k]
        out[core * BL:(core + 1) * BL] = (
            spk.transpose(2, 1, 3, 0).reshape(BL, S * T, H))
        vfc = r["vf"].reshape(128, BL, KB)              # [h', b, k]
        vf[core * BL:(core + 1) * BL] = (
            vfc.transpose(1, 2, 0).reshape(BL, H))
    return out, vf
